# revision 9
# baseline (speedup 1.0000x reference)
"""AttnBlock (GroupNorm -> QKV -> 4096x4096 single-head attention -> proj ->
residual) on 8 TRN2 NeuronCores.

Sharding: data-parallel over batch (B=2) x sequence-parallel over query
positions (4 slabs of 1024). Each core receives the full x[b] (rolled so its
query slab sits at columns 0:1024), computes GroupNorm stats + k/v for the
whole image (replicated within the 4-core batch group -> zero collectives),
and attention + projection + residual for its 1024 query columns only.

Precision: matmuls in bf16 with fp32 PSUM accumulation; GroupNorm stats,
softmax sums and the residual path in fp32.
"""
import sys
sys.path.insert(0, '/opt/trn_rl_repo')
import numpy as np
import ml_dtypes

import concourse.bass as bass
import concourse.tile as tile
from concourse import mybir, bacc
from concourse import bass_utils

f32 = mybir.dt.float32
bf16 = mybir.dt.bfloat16
AF = mybir.ActivationFunctionType
ALU = mybir.AluOpType

C = 512          # channels
N = 4096         # positions (64*64)
G = 32           # groupnorm groups
GP = 16          # channels per group
NT = C // 128    # 4 channel partition-tiles
QS = 1024        # query slab per core
EPS = 1e-6
NELEM = float(GP * N)   # elements per group


def _patch_act_tables():
    # Reorder so the set containing every function we use (Square, Ln, Exp,
    # Identity, Copy) is picked for all of them -> exactly one table load.
    import concourse.bacc as _bacc_mod
    from concourse import hw_specs as _hw
    if getattr(_bacc_mod, "_act_tables_patched", False):
        return
    _orig = _hw.get_activation_tables

    def _patched(arch):
        t = dict(_orig(arch))
        key = "natural_log_exp_and_others"
        if key in t:
            order = [key] + [k for k in t if k != key]
            t = {k: t[k] for k in order}
        return t

    # DISABLED: reordering shifts act_func_set_id out of sync with walrus's
    # act_info.json indexing -> wrong spline tables (NaN). Keep identity.
    _bacc_mod._act_tables_patched = True


def _build():
    _patch_act_tables()
    nc = bacc.Bacc("TRN2", target_bir_lowering=False, debug=False, num_devices=8)

    # xbf: bf16 image [C, N] (rolled); xslab: fp32 residual slab [C, QS]
    # wall: packed bf16 weights [128, 16*512], block (w*4+t) = wT_w rows t*128..
    # misc: packed fp32 small inputs [128, 668]
    xbf_d = nc.dram_tensor("xbf", [C, N], bf16, kind="ExternalInput")
    xslab_d = nc.dram_tensor("xslab", [C, QS], f32, kind="ExternalInput")
    wall_d = nc.dram_tensor("wall", [128, 16 * 512], bf16, kind="ExternalInput")
    misc_d = nc.dram_tensor("misc", [128, 668], f32, kind="ExternalInput")
    out = nc.dram_tensor("out", [C, QS], f32, kind="ExternalOutput")

    with tile.TileContext(nc) as tc:
        import contextlib
        with contextlib.ExitStack() as ctx:
            singles = ctx.enter_context(tc.tile_pool(name="singles", bufs=1))
            rot = ctx.enter_context(tc.tile_pool(name="rot", bufs=3))
            sq_scr = ctx.enter_context(tc.tile_pool(name="sq_scr", bufs=2))
            xbf = ctx.enter_context(tc.tile_pool(name="xbf", bufs=1))
            xslab = ctx.enter_context(tc.tile_pool(name="xslab", bufs=1))
            ksb = ctx.enter_context(tc.tile_pool(name="ksb", bufs=1))
            vtsb = ctx.enter_context(tc.tile_pool(name="vtsb", bufs=1))
            qsb = ctx.enter_context(tc.tile_pool(name="qsb", bufs=1))
            psb = ctx.enter_context(tc.tile_pool(name="psb", bufs=3))
            onsb = ctx.enter_context(tc.tile_pool(name="onsb", bufs=8))
            outsb = ctx.enter_context(tc.tile_pool(name="outsb", bufs=2))
            small = ctx.enter_context(tc.tile_pool(name="small", bufs=1))
            ps_acc = ctx.enter_context(tc.tile_pool(name="ps_acc", bufs=1, space="PSUM"))
            ps_st = ctx.enter_context(tc.tile_pool(name="ps_st", bufs=3, space="PSUM"))

            # ---- phase 0: x chunks first (critical path), then packed inputs
            x_bf = [xbf.tile([128, N], bf16, name=f"xbf{t}", tag=f"xbf{t}") for t in range(NT)]
            sum_parts = small.tile([128, 8], f32, tag="sum_parts")
            sq_parts = small.tile([128, 8], f32, tag="sq_parts")
            xchunk_dmas = []
            for i in range(8):
                t, j = i // 2, i % 2
                sl = x_bf[t][:, j * 2048:(j + 1) * 2048]
                nc.sync.dma_start(sl, xbf_d.ap()[t * 128:(t + 1) * 128,
                                                 j * 2048:(j + 1) * 2048])
                xchunk_dmas.append(sl)
            wall_t = singles.tile([128, 16 * 512], bf16, tag="wall")
            nc.sync.dma_start(wall_t[:], wall_d.ap())
            misc_t = singles.tile([128, 668], f32, tag="misc")
            nc.sync.dma_start(misc_t[:], misc_d.ap())
            x_sl = []
            for t in range(NT):
                xs = xslab.tile([128, QS], f32, tag=f"xsl{t}")
                nc.sync.dma_start(xs[:], xslab_d.ap()[t * 128:(t + 1) * 128, :])
                x_sl.append(xs)

            # views into the packed tiles
            w_t = {}
            for iw, name in enumerate(("q", "k", "v", "p")):
                w_t[name] = [wall_t[:, (iw * 4 + t) * 512:(iw * 4 + t + 1) * 512]
                             for t in range(NT)]
            bqk_t = [misc_t[:, 2 * t:2 * t + 2] for t in range(NT)]
            bp_t = [misc_t[:, 8 + t:9 + t] for t in range(NT)]
            gnw_t = [misc_t[:, 12 + t:13 + t] for t in range(NT)]
            gnb_t = [misc_t[:, 16 + t:17 + t] for t in range(NT)]
            sel8_t = misc_t[:, 20:28]
            sel8T_t = misc_t[0:8, 28:156]
            bvr = misc_t[0:1, 156:668]

            ones_col = singles.tile([128, 1], bf16, tag="ones_col")
            nc.vector.memset(ones_col[:], 1.0)
            ones_row = singles.tile([1, 128], bf16, tag="ones_row")
            nc.vector.memset(ones_row[:], 1.0)
            eps8 = singles.tile([8, 1], f32, tag="eps8")
            nc.vector.memset(eps8[:], EPS)

            # stats per chunk (bf16 input, fp32 accumulation)
            for i in range(8):
                sl = xchunk_dmas[i]
                nc.vector.reduce_sum(sum_parts[:, i:i + 1], sl,
                                     axis=mybir.AxisListType.X)
                sq = sq_scr.tile([128, 2048], bf16, tag="sq")
                nc.scalar.activation(sq[:], sl, AF.Square,
                                     accum_out=sq_parts[:, i:i + 1])

            # ---- phase 0c: finalize group stats --------------------------
            # stats_both: cols 0-3 = per-(partition,ctile) sum, 4-7 = sumsq
            stats_both = small.tile([128, 8], f32, tag="stats_both")
            nc.vector.reduce_sum(stats_both[:, 0:4],
                                 sum_parts[:].rearrange("p (t j) -> p t j", j=2),
                                 axis=mybir.AxisListType.X)
            nc.vector.reduce_sum(stats_both[:, 4:8],
                                 sq_parts[:].rearrange("p (t j) -> p t j", j=2),
                                 axis=mybir.AxisListType.X)
            # group-reduce over the 16-channel groups: psum_g[8, 2t:2t+2]
            ps_g = ps_st.tile([8, 8], f32, tag="st")
            for t in range(NT):
                nc.tensor.matmul(ps_g[:, 2 * t:2 * t + 2], sel8_t,
                                 stats_both[:, t::4], start=True, stop=True)
            mstats = small.tile([8, 8], f32, tag="mstats")
            nc.scalar.mul(mstats[:], ps_g[:], 1.0 / NELEM)  # cols 2t: mean, 2t+1: E[x^2]
            mean_v = mstats[:, 0::2]
            ex2_v = mstats[:, 1::2]
            var8 = small.tile([8, 4], f32, tag="var8")
            m2 = small.tile([8, 4], f32, tag="m2")
            nc.vector.tensor_tensor(m2[:], mean_v, mean_v, op=ALU.mult)
            nc.vector.tensor_tensor(var8[:], ex2_v, m2[:], op=ALU.subtract)
            lnv = small.tile([8, 4], f32, tag="lnv")
            nc.scalar.activation(lnv[:], var8[:], AF.Ln, bias=eps8[:])
            rstd8 = small.tile([8, 4], f32, tag="rstd8")
            nc.scalar.activation(rstd8[:], lnv[:], AF.Exp, scale=-0.5)
            grp2 = small.tile([8, 8], f32, tag="grp2")
            nc.vector.tensor_copy(grp2[:, 0::2], mean_v)
            nc.vector.tensor_copy(grp2[:, 1::2], rstd8[:])
            # broadcast group -> channel: bc[128, 2] = (mean_c, rstd_c)
            scale_t, shift_bf = [], []
            for t in range(NT):
                ps_bc = ps_st.tile([128, 2], f32, tag="st")
                nc.tensor.matmul(ps_bc[:], sel8T_t, grp2[0:8, 2 * t:2 * t + 2],
                                 start=True, stop=True)
                sc = small.tile([128, 1], f32, tag=f"scale{t}")
                nc.vector.tensor_tensor(sc[:], gnw_t[t], ps_bc[:, 1:2], op=ALU.mult)
                scale_t.append(sc)
                nsc = small.tile([128, 1], f32, tag=f"nscale{t}")
                nc.vector.tensor_scalar_mul(nsc[:], sc[:], -1.0)
                sh = small.tile([128, 1], f32, tag=f"shift{t}")
                nc.vector.scalar_tensor_tensor(sh[:], ps_bc[:, 0:1], nsc[:],
                                               gnb_t[t], op0=ALU.mult, op1=ALU.add)
                shb = small.tile([128, 1], bf16, tag=f"shiftb{t}")
                nc.vector.tensor_copy(shb[:], sh[:])
                shift_bf.append(shb)

            # ---- phase 0d: fold GN into weights and biases ---------------
            # bias folds first (use unscaled weights)
            b2qk_t = []
            for t in range(NT):
                ps_b = ps_st.tile([128, 2], f32, tag="st")
                for j, wname in enumerate(("q", "k")):
                    for ct in range(NT):
                        nc.tensor.matmul(ps_b[:, j:j + 1],
                                         w_t[wname][ct][:, t * 128:(t + 1) * 128],
                                         shift_bf[ct][:],
                                         start=(ct == 0), stop=(ct == 3))
                b2 = small.tile([128, 2], f32, tag=f"b2qk{t}")
                nc.vector.tensor_tensor(b2[:], ps_b[:], bqk_t[t], op=ALU.add)
                b2qk_t.append(b2)
            ps_vr = ps_st.tile([1, C], f32, tag="st")
            for ct in range(NT):
                nc.tensor.matmul(ps_vr[:], shift_bf[ct][:], w_t["v"][ct],
                                 start=(ct == 0), stop=(ct == 3))
            bv2 = small.tile([1, C], f32, tag="bv2")
            nc.vector.tensor_tensor(bv2[:], ps_vr[:], bvr, op=ALU.add)
            bv2b = small.tile([1, C], bf16, tag="bv2b")
            nc.vector.tensor_copy(bv2b[:], bv2[:])
            # scale folds (in place on the weight tiles)
            for wname in ("q", "k", "v"):
                for ct in range(NT):
                    nc.vector.tensor_scalar_mul(w_t[wname][ct], w_t[wname][ct],
                                                scale_t[ct][:])

            # ---- phase 1: q, k, vT projections ---------------------------
            q_sb = [qsb.tile([128, QS], bf16, name=f"q{t}", tag=f"q{t}") for t in range(NT)]
            for t in range(NT):
                for nch in range(QS // 512):
                    ps = ps_st.tile([128, 512], f32, tag="st")
                    for ct in range(NT):
                        nc.tensor.matmul(ps[:],
                                         w_t["q"][ct][:, t * 128:(t + 1) * 128],
                                         x_bf[ct][:, nch * 512:(nch + 1) * 512],
                                         start=(ct == 0), stop=(ct == 3))
                    nc.scalar.activation(q_sb[t][:, nch * 512:(nch + 1) * 512],
                                         ps[:], AF.Identity, bias=b2qk_t[t][:, 0:1])
            k_sb = [ksb.tile([128, N], bf16, name=f"k{t}", tag=f"k{t}") for t in range(NT)]
            for t in range(NT):
                for nch in range(N // 512):
                    ps = ps_st.tile([128, 512], f32, tag="st")
                    for ct in range(NT):
                        nc.tensor.matmul(ps[:],
                                         w_t["k"][ct][:, t * 128:(t + 1) * 128],
                                         x_bf[ct][:, nch * 512:(nch + 1) * 512],
                                         start=(ct == 0), stop=(ct == 3))
                    nc.scalar.activation(k_sb[t][:, nch * 512:(nch + 1) * 512],
                                         ps[:], AF.Identity, bias=b2qk_t[t][:, 1:2])
            vt_sb = [vtsb.tile([128, C], bf16, name=f"vt{nt}", tag=f"vt{nt}") for nt in range(N // 128)]
            for nt in range(N // 128):
                ps = ps_st.tile([128, C], f32, tag="st")
                for ct in range(NT):
                    nc.tensor.matmul(ps[:],
                                     x_bf[ct][:, nt * 128:(nt + 1) * 128],
                                     w_t["v"][ct],
                                     start=(ct == 0), stop=False)
                nc.tensor.matmul(ps[:], ones_row[:], bv2b[:], start=False, stop=True)
                nc.vector.tensor_copy(vt_sb[nt][:], ps[:])

            # ---- phase 2: attention + projection per 512-query chunk -----
            for qch in range(QS // 512):
                o_ps = [ps_acc.tile([128, 512], f32, name=f"o{t}", tag=f"o{t}") for t in range(NT)]
                sums_ps = ps_acc.tile([1, 512], f32, tag="sums")
                for kt in range(N // 128):
                    st_ps = ps_st.tile([128, 512], f32, tag="st")
                    for ct in range(NT):
                        nc.tensor.matmul(st_ps[:],
                                         k_sb[ct][:, kt * 128:(kt + 1) * 128],
                                         q_sb[ct][:, qch * 512:(qch + 1) * 512],
                                         start=(ct == 0), stop=(ct == 3))
                    p_t = psb.tile([128, 512], bf16, tag="p")
                    nc.scalar.activation(p_t[:], st_ps[:], AF.Exp)
                    for ct in range(NT):
                        nc.tensor.matmul(o_ps[ct][:],
                                         vt_sb[kt][:, ct * 128:(ct + 1) * 128],
                                         p_t[:],
                                         start=(kt == 0), stop=(kt == N // 128 - 1))
                    nc.tensor.matmul(sums_ps[:], ones_col[:], p_t[:],
                                     start=(kt == 0), stop=(kt == N // 128 - 1))
                o_n = []
                for ct in range(NT):
                    on = onsb.tile([128, 512], bf16, tag="on")
                    nc.vector.tensor_copy(on[:], o_ps[ct][:])
                    o_n.append(on)
                r_row = small.tile([1, 512], f32, tag="r_row")
                nc.vector.reciprocal(r_row[:], sums_ps[:])
                r_bf = small.tile([1, 512], bf16, tag="r_bf")
                nc.vector.tensor_copy(r_bf[:], r_row[:])
                ps_rb = ps_st.tile([128, 512], f32, tag="st")
                nc.tensor.matmul(ps_rb[:], ones_row[:], r_bf[:], start=True, stop=True)
                r_all = small.tile([128, 512], f32, tag="r_all")
                nc.scalar.copy(r_all[:], ps_rb[:])
                for t in range(NT):
                    pp = ps_acc.tile([128, 512], f32, tag=f"o{t}")
                    for ct in range(NT):
                        nc.tensor.matmul(pp[:],
                                         w_t["p"][ct][:, t * 128:(t + 1) * 128],
                                         o_n[ct][:],
                                         start=(ct == 0), stop=(ct == 3))
                    t1 = outsb.tile([128, 512], f32, tag="t1")
                    nc.vector.tensor_tensor(t1[:], pp[:], r_all[:], op=ALU.mult)
                    ot = outsb.tile([128, 512], f32, tag="ot")
                    nc.vector.scalar_tensor_tensor(
                        ot[:], t1[:], bp_t[t],
                        x_sl[t][:, qch * 512:(qch + 1) * 512],
                        op0=ALU.add, op1=ALU.add)
                    nc.sync.dma_start(
                        out.ap()[t * 128:(t + 1) * 128, qch * 512:(qch + 1) * 512],
                        ot[:])
    nc.compile()
    return nc


_NC = None


def _get_nc():
    global _NC
    if _NC is None:
        _NC = _build()
    return _NC


def kernel(x, gn_w, gn_b, wq, bq, wk, bk, wv, bv, wp, bp):
    x = np.asarray(x, dtype=np.float32)
    B = x.shape[0]
    assert x.shape == (B, C, 64, 64)
    scale = float(C) ** -0.5

    wqT = np.ascontiguousarray((np.asarray(wq, np.float32) * scale).T
                               ).astype(ml_dtypes.bfloat16)
    wkT = np.ascontiguousarray(np.asarray(wk, np.float32).T).astype(ml_dtypes.bfloat16)
    wvT = np.ascontiguousarray(np.asarray(wv, np.float32).T).astype(ml_dtypes.bfloat16)
    wpT = np.ascontiguousarray(np.asarray(wp, np.float32).T).astype(ml_dtypes.bfloat16)
    wall = np.zeros((128, 16 * 512), ml_dtypes.bfloat16)
    for iw, wT in enumerate((wqT, wkT, wvT, wpT)):
        for t in range(NT):
            wall[:, (iw * 4 + t) * 512:(iw * 4 + t + 1) * 512] = \
                wT[t * 128:(t + 1) * 128, :]

    misc = np.zeros((128, 668), np.float32)
    bq_s = np.asarray(bq, np.float32) * scale
    bk_a = np.asarray(bk, np.float32)
    bp_a = np.asarray(bp, np.float32)
    gnw_a = np.asarray(gn_w, np.float32)
    gnb_a = np.asarray(gn_b, np.float32)
    for t in range(NT):
        sl = slice(t * 128, (t + 1) * 128)
        misc[:, 2 * t] = bq_s[sl]
        misc[:, 2 * t + 1] = bk_a[sl]
        misc[:, 8 + t] = bp_a[sl]
        misc[:, 12 + t] = gnw_a[sl]
        misc[:, 16 + t] = gnb_a[sl]
    sel8 = np.zeros((128, 8), np.float32)
    for p in range(128):
        sel8[p, p // GP] = 1.0
    misc[:, 20:28] = sel8
    misc[0:8, 28:156] = sel8.T
    misc[0:1, 156:668] = np.asarray(bv, np.float32).reshape(1, C)

    xf = x.reshape(B, C, N)
    in_maps = []
    for core in range(8):
        b, slab = core // 4, core % 4
        xr = np.roll(xf[b], -QS * slab, axis=1)
        in_maps.append({
            "xbf": np.ascontiguousarray(xr).astype(ml_dtypes.bfloat16),
            "xslab": np.ascontiguousarray(xr[:, 0:QS]),
            "wall": wall, "misc": misc,
        })

    global _last_in_maps
    _last_in_maps = in_maps
    nc = _get_nc()
    res = bass_utils.run_bass_kernel_spmd(nc, in_maps, core_ids=list(range(8)))

    out = np.empty((B, C, N), np.float32)
    for core in range(8):
        b, slab = core // 4, core % 4
        out[b][:, QS * slab:QS * (slab + 1)] = res.results[core]["out"]
    return out.reshape(B, C, 64, 64)


if __name__ == "__main__":
    rng = np.random.default_rng(0)
    inputs = {
        "x": rng.standard_normal((2, C, 64, 64)).astype(np.float32),
        "gn_w": np.ones(C, np.float32),
        "gn_b": np.zeros(C, np.float32),
    }
    for nm in ("q", "k", "v", "p"):
        inputs[f"w{nm}"] = (rng.standard_normal((C, C)) * 0.02).astype(np.float32)
        inputs[f"b{nm}"] = np.zeros(C, np.float32)
    out = kernel(**inputs)
    print("ran:", out.shape, out.dtype)


# revision 12
# speedup vs baseline: 29.0641x; 29.0641x over previous
"""AttnBlock (GroupNorm -> QKV -> 4096x4096 single-head attention -> proj ->
residual) on 8 TRN2 NeuronCores.

Sharding: data-parallel over batch (B=2) x sequence-parallel over query
positions (4 slabs of 1024). Each core receives the full x[b] (rolled so its
query slab sits at columns 0:1024), computes GroupNorm stats + k/v for the
whole image (replicated within the 4-core batch group -> zero collectives),
and attention + projection + residual for its 1024 query columns only.

Precision: matmuls in bf16 with fp32 PSUM accumulation; GroupNorm stats,
softmax sums and the residual path in fp32.
"""
import sys
sys.path.insert(0, '/opt/trn_rl_repo')
import contextlib
import numpy as np
import ml_dtypes

import concourse.bass as bass
import concourse.tile as tile
from concourse import mybir, bacc
from concourse import bass_utils

f32 = mybir.dt.float32
bf16 = mybir.dt.bfloat16
AF = mybir.ActivationFunctionType
ALU = mybir.AluOpType

C = 512          # channels
N = 4096         # positions (64*64)
G = 32           # groupnorm groups
GP = 16          # channels per group
NT = C // 128    # 4 channel partition-tiles
QS = 1024        # query slab per core
EPS = 1e-6
NELEM = float(GP * N)   # elements per group


def _make_pools(tc, ctx):
    p = {}
    p["singles"] = ctx.enter_context(tc.tile_pool(name="singles", bufs=1))
    p["sq_scr"] = ctx.enter_context(tc.tile_pool(name="sq_scr", bufs=2))
    p["xbf"] = ctx.enter_context(tc.tile_pool(name="xbf", bufs=1))
    p["xslab"] = ctx.enter_context(tc.tile_pool(name="xslab", bufs=1))
    p["ksb"] = ctx.enter_context(tc.tile_pool(name="ksb", bufs=1))
    p["vtsb"] = ctx.enter_context(tc.tile_pool(name="vtsb", bufs=1))
    p["qsb"] = ctx.enter_context(tc.tile_pool(name="qsb", bufs=1))
    p["psb"] = ctx.enter_context(tc.tile_pool(name="psb", bufs=3))
    p["onsb"] = ctx.enter_context(tc.tile_pool(name="onsb", bufs=8))
    p["outsb"] = ctx.enter_context(tc.tile_pool(name="outsb", bufs=2))
    p["small"] = ctx.enter_context(tc.tile_pool(name="small", bufs=1))
    p["ps_acc"] = ctx.enter_context(tc.tile_pool(name="ps_acc", bufs=1, space="PSUM"))
    p["ps_st"] = ctx.enter_context(tc.tile_pool(name="ps_st", bufs=3, space="PSUM"))
    return p


def _emit_body(nc, tc, p, xbf_d, xslab_d, wall_d, misc_d, out):
    singles, sq_scr, xbf, xslab = p["singles"], p["sq_scr"], p["xbf"], p["xslab"]
    ksb, vtsb, qsb, psb = p["ksb"], p["vtsb"], p["qsb"], p["psb"]
    onsb, outsb, small = p["onsb"], p["outsb"], p["small"]
    ps_acc, ps_st = p["ps_acc"], p["ps_st"]

    # ---- phase 0: x chunks first (critical path), then packed inputs
    x_bf = [xbf.tile([128, N], bf16, name=f"xbf{t}", tag=f"xbf{t}")
            for t in range(NT)]
    sum_parts = small.tile([128, 8], f32, tag="sum_parts")
    sq_parts = small.tile([128, 8], f32, tag="sq_parts")
    xchunk = []
    for i in range(8):
        t, j = i // 2, i % 2
        sl = x_bf[t][:, j * 2048:(j + 1) * 2048]
        nc.sync.dma_start(sl, xbf_d.ap()[t * 128:(t + 1) * 128,
                                         j * 2048:(j + 1) * 2048])
        xchunk.append(sl)
    wall_t = singles.tile([128, 16 * 512], bf16, tag="wall")
    nc.sync.dma_start(wall_t[:], wall_d.ap())
    misc_t = singles.tile([128, 668], f32, tag="misc")
    nc.sync.dma_start(misc_t[:], misc_d.ap())
    x_sl = []
    for t in range(NT):
        xs = xslab.tile([128, QS], f32, tag=f"xsl{t}")
        nc.sync.dma_start(xs[:], xslab_d.ap()[t * 128:(t + 1) * 128, :])
        x_sl.append(xs)

    # views into the packed tiles
    w_t = {}
    for iw, name in enumerate(("q", "k", "v", "p")):
        w_t[name] = [wall_t[:, (iw * 4 + t) * 512:(iw * 4 + t + 1) * 512]
                     for t in range(NT)]
    bqk_t = [misc_t[:, 2 * t:2 * t + 2] for t in range(NT)]
    bp_t = [misc_t[:, 8 + t:9 + t] for t in range(NT)]
    gnw_t = [misc_t[:, 12 + t:13 + t] for t in range(NT)]
    gnb_t = [misc_t[:, 16 + t:17 + t] for t in range(NT)]
    sel8_t = misc_t[:, 20:28]
    sel8T_t = misc_t[0:8, 28:156]
    bvr = misc_t[0:1, 156:668]

    ones_col = singles.tile([128, 1], bf16, tag="ones_col")
    nc.vector.memset(ones_col[:], 1.0)
    ones_row = singles.tile([1, 128], bf16, tag="ones_row")
    nc.vector.memset(ones_row[:], 1.0)
    eps8 = singles.tile([8, 1], f32, tag="eps8")
    nc.vector.memset(eps8[:], EPS)

    # stats per chunk (bf16 input, fp32 accumulation)
    for i in range(8):
        sl = xchunk[i]
        nc.vector.reduce_sum(sum_parts[:, i:i + 1], sl,
                             axis=mybir.AxisListType.X)
        sq = sq_scr.tile([128, 2048], bf16, tag="sq")
        nc.scalar.activation(sq[:], sl, AF.Square,
                             accum_out=sq_parts[:, i:i + 1])

    # ---- phase 0c: finalize group stats --------------------------
    stats_both = small.tile([128, 8], f32, tag="stats_both")
    nc.vector.reduce_sum(stats_both[:, 0:4],
                         sum_parts[:].rearrange("p (t j) -> p t j", j=2),
                         axis=mybir.AxisListType.X)
    nc.vector.reduce_sum(stats_both[:, 4:8],
                         sq_parts[:].rearrange("p (t j) -> p t j", j=2),
                         axis=mybir.AxisListType.X)
    # group-reduce over the 16-channel groups: psum_g[8, 2t:2t+2]
    ps_g = ps_st.tile([8, 8], f32, tag="st")
    for t in range(NT):
        nc.tensor.matmul(ps_g[:, 2 * t:2 * t + 2], sel8_t,
                         stats_both[:, t::4], start=True, stop=True)
    mstats = small.tile([8, 8], f32, tag="mstats")
    nc.scalar.mul(mstats[:], ps_g[:], 1.0 / NELEM)
    mean_v = mstats[:, 0::2]
    ex2_v = mstats[:, 1::2]
    var8 = small.tile([8, 4], f32, tag="var8")
    m2 = small.tile([8, 4], f32, tag="m2")
    nc.vector.tensor_tensor(m2[:], mean_v, mean_v, op=ALU.mult)
    nc.vector.tensor_tensor(var8[:], ex2_v, m2[:], op=ALU.subtract)
    lnv = small.tile([8, 4], f32, tag="lnv")
    nc.scalar.activation(lnv[:], var8[:], AF.Ln, bias=eps8[:])
    rstd8 = small.tile([8, 4], f32, tag="rstd8")
    nc.scalar.activation(rstd8[:], lnv[:], AF.Exp, scale=-0.5)
    grp2 = small.tile([8, 8], f32, tag="grp2")
    nc.vector.tensor_copy(grp2[:, 0::2], mean_v)
    nc.vector.tensor_copy(grp2[:, 1::2], rstd8[:])
    # broadcast group -> channel: bc[128, 2] = (mean_c, rstd_c)
    scale_t, shift_bf = [], []
    for t in range(NT):
        ps_bc = ps_st.tile([128, 2], f32, tag="st")
        nc.tensor.matmul(ps_bc[:], sel8T_t, grp2[0:8, 2 * t:2 * t + 2],
                         start=True, stop=True)
        sc = small.tile([128, 1], f32, tag=f"scale{t}")
        nc.vector.tensor_tensor(sc[:], gnw_t[t], ps_bc[:, 1:2], op=ALU.mult)
        scale_t.append(sc)
        nsc = small.tile([128, 1], f32, tag=f"nscale{t}")
        nc.vector.tensor_scalar_mul(nsc[:], sc[:], -1.0)
        sh = small.tile([128, 1], f32, tag=f"shift{t}")
        nc.vector.scalar_tensor_tensor(sh[:], ps_bc[:, 0:1], nsc[:],
                                       gnb_t[t], op0=ALU.mult, op1=ALU.add)
        shb = small.tile([128, 1], bf16, tag=f"shiftb{t}")
        nc.vector.tensor_copy(shb[:], sh[:])
        shift_bf.append(shb)

    # ---- phase 0d: fold GN into weights and biases ---------------
    b2qk_t = []
    for t in range(NT):
        ps_b = ps_st.tile([128, 2], f32, tag="st")
        for j, wname in enumerate(("q", "k")):
            for ct in range(NT):
                nc.tensor.matmul(ps_b[:, j:j + 1],
                                 w_t[wname][ct][:, t * 128:(t + 1) * 128],
                                 shift_bf[ct][:],
                                 start=(ct == 0), stop=(ct == 3))
        b2 = small.tile([128, 2], f32, tag=f"b2qk{t}")
        nc.vector.tensor_tensor(b2[:], ps_b[:], bqk_t[t], op=ALU.add)
        b2qk_t.append(b2)
    ps_vr = ps_st.tile([1, C], f32, tag="st")
    for ct in range(NT):
        nc.tensor.matmul(ps_vr[:], shift_bf[ct][:], w_t["v"][ct],
                         start=(ct == 0), stop=(ct == 3))
    bv2 = small.tile([1, C], f32, tag="bv2")
    nc.vector.tensor_tensor(bv2[:], ps_vr[:], bvr, op=ALU.add)
    bv2b = small.tile([1, C], bf16, tag="bv2b")
    nc.vector.tensor_copy(bv2b[:], bv2[:])
    # broadcast v-bias across partitions once (vs a K=1 matmul per n-tile)
    ps_bb = ps_st.tile([128, C], f32, tag="st")
    nc.tensor.matmul(ps_bb[:], ones_row[:], bv2b[:], start=True, stop=True)
    bv_bc = singles.tile([128, C], bf16, tag="bv_bc")
    nc.scalar.copy(bv_bc[:], ps_bb[:])
    # scale folds (in place on the weight tiles)
    for wname in ("q", "k", "v"):
        for ct in range(NT):
            nc.vector.tensor_scalar_mul(w_t[wname][ct], w_t[wname][ct],
                                        scale_t[ct][:])

    # ---- phase 1: q, k, vT projections ---------------------------
    q_sb = [qsb.tile([128, QS], bf16, name=f"q{t}", tag=f"q{t}")
            for t in range(NT)]
    for t in range(NT):
        for nch in range(QS // 512):
            ps = ps_st.tile([128, 512], f32, tag="st")
            for ct in range(NT):
                nc.tensor.matmul(ps[:],
                                 w_t["q"][ct][:, t * 128:(t + 1) * 128],
                                 x_bf[ct][:, nch * 512:(nch + 1) * 512],
                                 start=(ct == 0), stop=(ct == 3))
            nc.scalar.activation(q_sb[t][:, nch * 512:(nch + 1) * 512],
                                 ps[:], AF.Identity, bias=b2qk_t[t][:, 0:1])
    k_sb = [ksb.tile([128, N], bf16, name=f"k{t}", tag=f"k{t}")
            for t in range(NT)]
    for t in range(NT):
        for nch in range(N // 512):
            ps = ps_st.tile([128, 512], f32, tag="st")
            for ct in range(NT):
                nc.tensor.matmul(ps[:],
                                 w_t["k"][ct][:, t * 128:(t + 1) * 128],
                                 x_bf[ct][:, nch * 512:(nch + 1) * 512],
                                 start=(ct == 0), stop=(ct == 3))
            nc.scalar.activation(k_sb[t][:, nch * 512:(nch + 1) * 512],
                                 ps[:], AF.Identity, bias=b2qk_t[t][:, 1:2])
    vt_sb = [vtsb.tile([128, C], bf16, name=f"vt{nt}", tag=f"vt{nt}")
             for nt in range(N // 128)]
    for nt in range(N // 128):
        ps = ps_st.tile([128, C], f32, tag="st")
        for ct in range(NT):
            nc.tensor.matmul(ps[:],
                             x_bf[ct][:, nt * 128:(nt + 1) * 128],
                             w_t["v"][ct],
                             start=(ct == 0), stop=(ct == NT - 1))
        nc.vector.tensor_tensor(vt_sb[nt][:], ps[:], bv_bc[:], op=ALU.add)

    # ---- phase 2: attention + projection per 512-query chunk -----
    for qch in range(QS // 512):
        o_ps = [ps_acc.tile([128, 512], f32, name=f"o{t}", tag=f"o{t}")
                for t in range(NT)]
        sums_ps = ps_acc.tile([1, 512], f32, tag="sums")
        for kt in range(N // 128):
            st_ps = ps_st.tile([128, 512], f32, tag="st")
            for ct in range(NT):
                nc.tensor.matmul(st_ps[:],
                                 k_sb[ct][:, kt * 128:(kt + 1) * 128],
                                 q_sb[ct][:, qch * 512:(qch + 1) * 512],
                                 start=(ct == 0), stop=(ct == 3))
            p_t = psb.tile([128, 512], bf16, tag="p")
            nc.scalar.activation(p_t[:], st_ps[:], AF.Exp)
            for ct in range(NT):
                nc.tensor.matmul(o_ps[ct][:],
                                 vt_sb[kt][:, ct * 128:(ct + 1) * 128],
                                 p_t[:],
                                 start=(kt == 0), stop=(kt == N // 128 - 1))
            nc.tensor.matmul(sums_ps[:], ones_col[:], p_t[:],
                             start=(kt == 0), stop=(kt == N // 128 - 1))
        o_n = []
        for ct in range(NT):
            on = onsb.tile([128, 512], bf16, tag="on")
            nc.vector.tensor_copy(on[:], o_ps[ct][:])
            o_n.append(on)
        r_row = small.tile([1, 512], f32, tag="r_row")
        nc.vector.reciprocal(r_row[:], sums_ps[:])
        r_bf = small.tile([1, 512], bf16, tag="r_bf")
        nc.vector.tensor_copy(r_bf[:], r_row[:])
        ps_rb = ps_st.tile([128, 512], f32, tag="st")
        nc.tensor.matmul(ps_rb[:], ones_row[:], r_bf[:], start=True, stop=True)
        r_all = small.tile([128, 512], f32, tag="r_all")
        nc.scalar.copy(r_all[:], ps_rb[:])
        for t in range(NT):
            pp = ps_acc.tile([128, 512], f32, tag=f"o{t}")
            for ct in range(NT):
                nc.tensor.matmul(pp[:],
                                 w_t["p"][ct][:, t * 128:(t + 1) * 128],
                                 o_n[ct][:],
                                 start=(ct == 0), stop=(ct == 3))
            t1 = outsb.tile([128, 512], f32, tag="t1")
            nc.vector.tensor_tensor(t1[:], pp[:], r_all[:], op=ALU.mult)
            ot = outsb.tile([128, 512], f32, tag="ot")
            nc.vector.scalar_tensor_tensor(
                ot[:], t1[:], bp_t[t],
                x_sl[t][:, qch * 512:(qch + 1) * 512],
                op0=ALU.add, op1=ALU.add)
            nc.sync.dma_start(
                out.ap()[t * 128:(t + 1) * 128, qch * 512:(qch + 1) * 512],
                ot[:])


def _build():
    nc = bacc.Bacc("TRN2", target_bir_lowering=False, debug=False, num_devices=8)
    xbf_d = nc.dram_tensor("xbf", [C, N], bf16, kind="ExternalInput")
    xslab_d = nc.dram_tensor("xslab", [C, QS], f32, kind="ExternalInput")
    wall_d = nc.dram_tensor("wall", [128, 16 * 512], bf16, kind="ExternalInput")
    misc_d = nc.dram_tensor("misc", [128, 668], f32, kind="ExternalInput")
    out = nc.dram_tensor("out", [C, QS], f32, kind="ExternalOutput")
    with tile.TileContext(nc) as tc:
        with contextlib.ExitStack() as ctx:
            p = _make_pools(tc, ctx)
            _emit_body(nc, tc, p, xbf_d, xslab_d, wall_d, misc_d, out)
    nc.compile()
    return nc


def _build_timing(reps):
    """Same body repeated `reps` times in a hardware loop; inputs live in
    internal DRAM (no host transfer) so per-call wall time differences
    isolate on-device execution."""
    nc = bacc.Bacc("TRN2", target_bir_lowering=False, debug=False, num_devices=8)
    xbf_d = nc.dram_tensor("xbf", [C, N], bf16, kind="ExternalInput")
    xslab_d = nc.dram_tensor("xslab", [C, QS], f32, kind="ExternalInput")
    wall_d = nc.dram_tensor("wall", [128, 16 * 512], bf16, kind="ExternalInput")
    misc_d = nc.dram_tensor("misc", [128, 668], f32, kind="ExternalInput")
    out = nc.dram_tensor("out", [C, QS], f32, kind="ExternalOutput")
    with tile.TileContext(nc) as tc:
        with contextlib.ExitStack() as ctx:
            p = _make_pools(tc, ctx)
            if reps == 1:
                _emit_body(nc, tc, p, xbf_d, xslab_d, wall_d, misc_d, out)
            else:
                with tc.For_i(0, reps, 1):
                    _emit_body(nc, tc, p, xbf_d, xslab_d, wall_d, misc_d, out)
    nc.compile()
    return nc


_NC = None


def _get_nc():
    global _NC
    if _NC is None:
        _NC = _build()
    return _NC


def kernel(x, gn_w, gn_b, wq, bq, wk, bk, wv, bv, wp, bp):
    x = np.asarray(x, dtype=np.float32)
    B = x.shape[0]
    assert x.shape == (B, C, 64, 64)
    scale = float(C) ** -0.5

    wqT = np.ascontiguousarray((np.asarray(wq, np.float32) * scale).T
                               ).astype(ml_dtypes.bfloat16)
    wkT = np.ascontiguousarray(np.asarray(wk, np.float32).T).astype(ml_dtypes.bfloat16)
    wvT = np.ascontiguousarray(np.asarray(wv, np.float32).T).astype(ml_dtypes.bfloat16)
    wpT = np.ascontiguousarray(np.asarray(wp, np.float32).T).astype(ml_dtypes.bfloat16)
    wall = np.zeros((128, 16 * 512), ml_dtypes.bfloat16)
    for iw, wT in enumerate((wqT, wkT, wvT, wpT)):
        for t in range(NT):
            wall[:, (iw * 4 + t) * 512:(iw * 4 + t + 1) * 512] = \
                wT[t * 128:(t + 1) * 128, :]

    misc = np.zeros((128, 668), np.float32)
    bq_s = np.asarray(bq, np.float32) * scale
    bk_a = np.asarray(bk, np.float32)
    bp_a = np.asarray(bp, np.float32)
    gnw_a = np.asarray(gn_w, np.float32)
    gnb_a = np.asarray(gn_b, np.float32)
    for t in range(NT):
        sl = slice(t * 128, (t + 1) * 128)
        misc[:, 2 * t] = bq_s[sl]
        misc[:, 2 * t + 1] = bk_a[sl]
        misc[:, 8 + t] = bp_a[sl]
        misc[:, 12 + t] = gnw_a[sl]
        misc[:, 16 + t] = gnb_a[sl]
    sel8 = np.zeros((128, 8), np.float32)
    for pp in range(128):
        sel8[pp, pp // GP] = 1.0
    misc[:, 20:28] = sel8
    misc[0:8, 28:156] = sel8.T
    misc[0:1, 156:668] = np.asarray(bv, np.float32).reshape(1, C)

    xf = x.reshape(B, C, N)
    in_maps = []
    for core in range(8):
        b, slab = core // 4, core % 4
        xr = np.roll(xf[b], -QS * slab, axis=1)
        in_maps.append({
            "xbf": np.ascontiguousarray(xr).astype(ml_dtypes.bfloat16),
            "xslab": np.ascontiguousarray(xr[:, 0:QS]),
            "wall": wall, "misc": misc,
        })

    global _last_in_maps
    _last_in_maps = in_maps
    nc = _get_nc()
    res = bass_utils.run_bass_kernel_spmd(nc, in_maps, core_ids=list(range(8)))

    out = np.empty((B, C, N), np.float32)
    for core in range(8):
        b, slab = core // 4, core % 4
        out[b][:, QS * slab:QS * (slab + 1)] = res.results[core]["out"]
    return out.reshape(B, C, 64, 64)


if __name__ == "__main__":
    rng = np.random.default_rng(0)
    inputs = {
        "x": rng.standard_normal((2, C, 64, 64)).astype(np.float32),
        "gn_w": np.ones(C, np.float32),
        "gn_b": np.zeros(C, np.float32),
    }
    for nm in ("q", "k", "v", "p"):
        inputs[f"w{nm}"] = (rng.standard_normal((C, C)) * 0.02).astype(np.float32)
        inputs[f"b{nm}"] = np.zeros(C, np.float32)
    out = kernel(**inputs)
    print("ran:", out.shape, out.dtype)
